# revision 1
# baseline (speedup 1.0000x reference)
"""GQA attention kernel for 8 trn2 NeuronCores.

Sharding: tensor-parallel over the 8 KV groups (1 group = 4 Q heads per
core, both batch elements), then an AllToAll reshards the per-core
context [256 feat, 4096 rows] into row-shards [2048 feat, 512 rows] so
the output projection runs row-parallel with no reduction.

Shapes (hardcoded): B=2, S=2048, D=2048, H=32, G=8, HD=64.
"""

import numpy as np
import concourse.bass as bass
import concourse.mybir as mybir
import concourse.tile as tile
from concourse import bacc
from concourse.bass import broadcast_tensor_aps
from concourse.bass_utils import run_bass_kernel_spmd
from concourse.masks import make_identity

N_CORES = 8
B, S, D = 2, 2048, 2048
H, G, HD = 32, 8, 64
GS = H // G                       # 4 q heads per kv group
ROWS = B * S                      # 4096 flattened (b, s) rows
RPC = ROWS // N_CORES             # 512 output rows per core
EPS = 1e-6
F32 = mybir.dt.float32
BF16 = mybir.dt.bfloat16
AX = mybir.AxisListType
ALU = mybir.AluOpType
AF = mybir.ActivationFunctionType

KB = D // 128                     # 16 contraction blocks for projections
MB = ROWS // 128                  # 32 row blocks
QKV = GS * HD + 2 * HD            # 384 projected features per core
NQK = GS + 1                      # 5 heads that get rmsnorm+rope (4 q + 1 k)
SQT = 512                         # attention query-tile width
SKT = 128                         # attention key-tile height
JQ = S // SQT                     # 4 query tiles per batch
IKB = S // SKT                    # 16 key blocks per batch


def _build():
    nc = bacc.Bacc(num_devices=N_CORES)

    xT = nc.dram_tensor("xT", [D, ROWS], BF16, kind="ExternalInput")
    wqkv = nc.dram_tensor("wqkv", [D, QKV], BF16, kind="ExternalInput")
    wo = nc.dram_tensor("wo", [H * HD, D], BF16, kind="ExternalInput")
    cs = nc.dram_tensor("cs", [S, HD], F32, kind="ExternalInput")
    sn = nc.dram_tensor("sn", [S, HD], F32, kind="ExternalInput")
    wvec = nc.dram_tensor("wvec", [1, NQK * HD], F32, kind="ExternalInput")
    maskM = nc.dram_tensor("maskM", [128, 1024], BF16, kind="ExternalInput")
    out_rows = nc.dram_tensor("out_rows", [RPC, D], F32, kind="ExternalOutput")

    with tile.TileContext(nc) as tc:
        with (
            tc.tile_pool(name="const", bufs=1) as const,
            tc.tile_pool(name="dram", bufs=1, space="DRAM") as dram,
        ):
            a2a_in01 = dram.tile([N_CORES, 2 * HD, RPC], BF16)
            a2a_out01 = dram.tile([N_CORES, 2 * HD, RPC], BF16)
            a2a_in23 = dram.tile([N_CORES, 2 * HD, RPC], BF16)
            a2a_out23 = dram.tile([N_CORES, 2 * HD, RPC], BF16)

            w_sb = const.tile([128, KB, QKV], BF16)
            nc.sync.dma_start(w_sb[:], wqkv[:].rearrange("(k p) j -> p k j", p=128))
            cos_sb = const.tile([128, S // 128, HD], F32)
            sin_sb = const.tile([128, S // 128, HD], F32)
            nc.sync.dma_start(cos_sb[:], cs[:].rearrange("(m p) d -> p m d", p=128))
            nc.sync.dma_start(sin_sb[:], sn[:].rearrange("(m p) d -> p m d", p=128))
            mask_sb = const.tile([128, 1024], BF16)
            nc.sync.dma_start(mask_sb[:], maskM[:])
            ident = const.tile([128, 128], F32)
            make_identity(nc, ident)
            wv1 = const.tile([1, NQK * HD], F32)
            nc.sync.dma_start(wv1[:], wvec[:])
            wv_sb = const.tile([128, NQK * HD], F32)
            nc.gpsimd.partition_broadcast(wv_sb[:], wv1[0:1, :])
            eps_sb = const.tile([128, 1], F32)
            nc.vector.memset(eps_sb[:], EPS)

            # persistent activations, split per batch so attention on
            # batch 0 can overlap projection of batch 1
            qT_a = [const.tile([128, S], BF16, name=f"qT_a{bb}") for bb in range(B)]
            qT_b = [const.tile([128, S], BF16, name=f"qT_b{bb}") for bb in range(B)]
            kT_t = [const.tile([128, S], BF16, name=f"kT{bb}") for bb in range(B)]
            v1_t = [const.tile([128, S // 128, 2 * HD], BF16, name=f"v1{bb}")
                    for bb in range(B)]
            for bb in range(B):
                nc.vector.memset(v1_t[bb][:], 1.0)  # cols 64:128 stay 1.0 (denominator)

            # attention PSUM pool first on the stack so it does not overlap
            # the projection pools (overlap would serialize the phases)
            with (
                tc.tile_pool(name="ps", bufs=3, space="PSUM") as pspool,
                tc.tile_pool(name="pc", bufs=2, space="PSUM") as pcpool,
                tc.tile_pool(name="xs", bufs=20) as xs,
                tc.tile_pool(name="ev", bufs=3) as ev,
                tc.tile_pool(name="ex", bufs=6) as ex,
                tc.tile_pool(name="cn", bufs=3) as cn,
                tc.tile_pool(name="cx", bufs=1) as cx,
                tc.tile_pool(name="ws", bufs=12) as ws,
                tc.tile_pool(name="ou", bufs=3) as ou,
            ):
                from contextlib import ExitStack
                pstack = ExitStack()
                ppool = pstack.enter_context(
                    tc.tile_pool(name="pp", bufs=2, space="PSUM"))
                tpool = pstack.enter_context(
                    tc.tile_pool(name="tp", bufs=1, space="PSUM"))
                for m4 in range(MB // 4):
                    xts = []
                    for k in range(KB):
                        t = xs.tile([128, 512], BF16, tag="xts")
                        nc.sync.dma_start(
                            t[:], xT[k * 128:(k + 1) * 128, m4 * 512:(m4 + 1) * 512]
                        )
                        xts.append(t)
                    for i in range(4):
                        m = m4 * 4 + i
                        pp = ppool.tile([128, QKV], F32, tag="pp")
                        for k in range(KB):
                            nc.tensor.matmul(
                                pp[:],
                                xts[k][:, i * 128:(i + 1) * 128],
                                w_sb[:, k, :],
                                start=(k == 0),
                                stop=(k == KB - 1),
                            )
                        # rmsnorm over each 64-wide head slice (q0..q3, k)
                        nqk = NQK * HD
                        sq = ev.tile([128, nqk], F32, tag="sq")
                        nc.scalar.activation(sq[:], pp[:, :nqk], AF.Square)
                        ssum = ev.tile([128, NQK], F32, tag="ssum")
                        nc.vector.tensor_reduce(
                            ssum[:], sq[:].rearrange("p (h d) -> p h d", d=HD),
                            AX.X, ALU.add,
                        )
                        srt = ev.tile([128, NQK], F32, tag="srt")
                        nc.scalar.activation(srt[:], ssum[:], AF.Sqrt,
                                             bias=eps_sb[:], scale=1.0 / HD)
                        rs = ev.tile([128, NQK], F32, tag="rs")
                        nc.vector.reciprocal(rs[:], srt[:])
                        qkn = ev.tile([128, nqk], F32, tag="qkn")
                        for h in range(NQK):
                            nc.vector.tensor_scalar_mul(
                                qkn[:, h * HD:(h + 1) * HD],
                                pp[:, h * HD:(h + 1) * HD],
                                rs[:, h:h + 1],
                            )
                        nc.vector.tensor_mul(qkn[:], qkn[:], wv_sb[:])
                        # rope (rotate-half) on all 5 heads at once
                        sm = m % (S // 128)
                        hf = HD // 2
                        qv = qkn[:].rearrange("p (h d) -> p h d", d=HD)
                        qkr = ev.tile([128, nqk], F32, tag="qkr")
                        rv = qkr[:].rearrange("p (h d) -> p h d", d=HD)
                        t1 = ev.tile([128, NQK, hf], F32, tag="t1")
                        t2 = ev.tile([128, NQK, hf], F32, tag="t2")

                        def bmul(out_ap, a_ap, trig, lo):
                            tr = trig[:, sm:sm + 1, lo * hf:(lo + 1) * hf]
                            a2, b2 = broadcast_tensor_aps(a_ap, tr)
                            nc.vector.tensor_tensor(out_ap, a2, b2, ALU.mult)

                        lo_in = qv[:, :, 0:hf]
                        hi_in = qv[:, :, hf:HD]
                        bmul(t1[:], hi_in, sin_sb, 0)        # x2 * sin_lo
                        bmul(t2[:], lo_in, sin_sb, 1)        # x1 * sin_hi
                        bmul(rv[:, :, 0:hf], lo_in, cos_sb, 0)
                        bmul(rv[:, :, hf:HD], hi_in, cos_sb, 1)
                        nc.vector.tensor_sub(rv[:, :, 0:hf], rv[:, :, 0:hf], t1[:])
                        nc.vector.tensor_add(rv[:, :, hf:HD], rv[:, :, hf:HD], t2[:])
                        # v straight from psum (no norm/rope)
                        bb, sm2 = m // (S // 128), m % (S // 128)
                        nc.vector.tensor_copy(v1_t[bb][:, sm2, 0:HD], pp[:, nqk:QKV])
                        # transposes: [seq,hd] -> [hd,seq]
                        tq1 = tpool.tile([128, 128], F32, tag="tq")
                        nc.tensor.transpose(tq1[:], qkr[:, 0:128], ident[:])
                        nc.vector.tensor_copy(qT_a[bb][:, sm2 * 128:(sm2 + 1) * 128], tq1[:])
                        tq2 = tpool.tile([128, 128], F32, tag="tq")
                        nc.tensor.transpose(tq2[:], qkr[:, 128:256], ident[:])
                        nc.vector.tensor_copy(qT_b[bb][:, sm2 * 128:(sm2 + 1) * 128], tq2[:])
                        kst = ev.tile([128, 128], F32, tag="kst")
                        nc.vector.tensor_copy(kst[:, 0:64], qkr[:, 256:320])
                        nc.vector.tensor_copy(kst[:, 64:128], qkr[:, 256:320])
                        tq3 = tpool.tile([128, 128], F32, tag="tq")
                        nc.tensor.transpose(tq3[:], kst[:], ident[:])
                        nc.vector.tensor_copy(kT_t[bb][:, sm2 * 128:(sm2 + 1) * 128], tq3[:])

                pstack.close()  # free proj PSUM banks for the out-proj pool
                popool_cm = tc.tile_pool(name="po", bufs=3, space="PSUM")
                popool = popool_cm.__enter__()
                o1p_cm = tc.tile_pool(name="o1p", bufs=16)
                o1p = o1p_cm.__enter__()

                # ------------ phase 2: attention (overlaps phase 1)
                for b in range(B):
                    for h in range(GS):
                        qT_t = qT_a[b] if h < 2 else qT_b[b]
                        hp = (h % 2) * 64
                        for jq in range(JQ):
                            q_rhs = qT_t[hp:hp + 64,
                                         jq * SQT:(jq + 1) * SQT]
                            pctx = pcpool.tile([2 * HD, SQT], F32, tag="pctx")
                            nkb = (jq + 1) * (SQT // SKT)
                            # chunks of 4 sk-blocks: emit 4 QKs, 4 exps, then
                            # 4 PVs so PE gets dense multi-matmul runs
                            for c0 in range(0, nkb, 4):
                                iks = range(c0, min(c0 + 4, nkb))
                                ess = []
                                for ik in iks:
                                    pss = pspool.tile([128, SQT], F32, tag="pss",
                                                      name=f"pss{ik}")
                                    k_lhs = kT_t[b][hp:hp + 64,
                                                  ik * SKT:(ik + 1) * SKT]
                                    nc.tensor.matmul(pss[:], k_lhs, q_rhs,
                                                     start=True, stop=True)
                                    es = ex.tile([128, SQT], BF16, tag="es",
                                                 name=f"es{ik}")
                                    nc.scalar.activation(es[:], pss[:], AF.Exp,
                                                         scale=1.0 / np.sqrt(HD))
                                    dd = ik * SKT - jq * SQT
                                    if dd >= 0:
                                        off = 512 - dd
                                        nc.vector.tensor_mul(
                                            es[:], es[:], mask_sb[:, off:off + SQT])
                                    ess.append(es)
                                for ik, es in zip(iks, ess):
                                    nc.tensor.matmul(
                                        pctx[:],
                                        v1_t[b][:, ik, :],
                                        es[:],
                                        start=(ik == 0),
                                        stop=(ik == nkb - 1),
                                    )
                            rinv = cn.tile([64, SQT], F32, tag="rinv")
                            nc.vector.reciprocal(rinv[:], pctx[HD:2 * HD, :])
                            ctxn = cn.tile([64, SQT], BF16, tag="ctxn")
                            nc.vector.tensor_mul(ctxn[:], pctx[0:HD, :], rinv[:])
                            a2a_dst = a2a_in01 if h < 2 else a2a_in23
                            hh = h % 2
                            nc.sync.dma_start(
                                a2a_dst[b * JQ + jq, hh * HD:(hh + 1) * HD, :],
                                ctxn[:])

                # ------------ phase 3: split all-to-all + row-parallel out-proj
                # CC#1 (heads 0,1) fires while attention on heads 2,3 still
                # runs; the even half of the out-projection overlaps too.
                nc.gpsimd.collective_compute(
                    "AllToAll", ALU.bypass,
                    replica_groups=[list(range(N_CORES))],
                    ins=[a2a_in01.opt()], outs=[a2a_out01.opt()])
                nc.gpsimd.collective_compute(
                    "AllToAll", ALU.bypass,
                    replica_groups=[list(range(N_CORES))],
                    ins=[a2a_in23.opt()], outs=[a2a_out23.opt()])
                flat01 = a2a_out01[:].rearrange("g f r -> (g f) r")
                flat23 = a2a_out23[:].rearrange("g f r -> (g f) r")
                cxt = {}
                for k in range(KB):
                    fl = flat01 if k % 2 == 0 else flat23
                    g = k // 2
                    t = cx.tile([128, RPC], BF16, tag=f"cx{k}", name=f"cx{k}")
                    nc.sync.dma_start(t[:], fl[g * 128:(g + 1) * 128, :])
                    cxt[k] = t
                NH = KB // 2
                o1s = {}
                for n in range(D // 512):
                    wts = []
                    for g in range(NH):
                        wt = ws.tile([128, 512], BF16, tag="wt", name=f"wte{n}_{g}")
                        k = 2 * g
                        nc.sync.dma_start(
                            wt[:], wo[k * 128:(k + 1) * 128, n * 512:(n + 1) * 512])
                        wts.append(wt)
                    for mi in range(4):
                        p1 = popool.tile([128, 512], F32, tag="po",
                                         name=f"p1_{n}_{mi}")
                        for g in range(NH):
                            nc.tensor.matmul(
                                p1[:], cxt[2 * g][:, mi * 128:(mi + 1) * 128],
                                wts[g][:], start=(g == 0), stop=(g == NH - 1))
                        o1 = o1p.tile([128, 512], F32, tag="o1", name=f"o1_{n}_{mi}")
                        nc.vector.tensor_copy(o1[:], p1[:])
                        o1s[(n, mi)] = o1
                for n in range(D // 512):
                    wts = []
                    for g in range(NH):
                        wt = ws.tile([128, 512], BF16, tag="wt", name=f"wto{n}_{g}")
                        k = 2 * g + 1
                        nc.sync.dma_start(
                            wt[:], wo[k * 128:(k + 1) * 128, n * 512:(n + 1) * 512])
                        wts.append(wt)
                    for mi in range(4):
                        p2 = popool.tile([128, 512], F32, tag="po",
                                         name=f"p2_{n}_{mi}")
                        for g in range(NH):
                            nc.tensor.matmul(
                                p2[:], cxt[2 * g + 1][:, mi * 128:(mi + 1) * 128],
                                wts[g][:], start=(g == 0), stop=(g == NH - 1))
                        ot = ou.tile([128, 512], F32, tag="ot", name=f"ot{n}_{mi}")
                        nc.vector.tensor_add(ot[:], p2[:], o1s[(n, mi)][:])
                        nc.sync.dma_start(
                            out_rows[mi * 128:(mi + 1) * 128,
                                     n * 512:(n + 1) * 512],
                            ot[:])
                popool_cm.__exit__(None, None, None)
                o1p_cm.__exit__(None, None, None)

    nc.finalize()
    return nc


_NC_CACHE = None


def _get_nc():
    global _NC_CACHE
    if _NC_CACHE is None:
        _NC_CACHE = _build()
    return _NC_CACHE


def _host_prep(x, cos, sin, Wq, Wk, Wv, Wo, q_norm_w, k_norm_w):
    import ml_dtypes
    BF = ml_dtypes.bfloat16
    xT = np.ascontiguousarray(
        np.asarray(x, np.float32).transpose(2, 0, 1).reshape(D, ROWS).astype(BF))
    f = np.arange(1024)[None, :]
    p = np.arange(128)[:, None]
    maskM = (p + 512 <= f).astype(BF)
    wvec = np.concatenate(
        [np.tile(np.asarray(q_norm_w, np.float32), GS),
         np.asarray(k_norm_w, np.float32)]).reshape(1, NQK * HD)
    base = dict(
        cs=np.ascontiguousarray(np.asarray(cos, np.float32)),
        sn=np.ascontiguousarray(np.asarray(sin, np.float32)),
        maskM=maskM,
        wvec=np.ascontiguousarray(wvec),
        xT=xT,
    )
    wo_c = np.ascontiguousarray(np.asarray(Wo, np.float32).astype(BF))
    in_maps = []
    for c in range(N_CORES):
        wqkv = np.concatenate(
            [np.asarray(Wq, np.float32)[:, c * GS * HD:(c + 1) * GS * HD],
             np.asarray(Wk, np.float32)[:, c * HD:(c + 1) * HD],
             np.asarray(Wv, np.float32)[:, c * HD:(c + 1) * HD]], axis=1)
        in_maps.append(dict(base, wqkv=np.ascontiguousarray(wqkv.astype(BF)),
                            wo=wo_c))
    return in_maps


def kernel(x, mask, cos, sin, Wq, Wk, Wv, Wo, q_norm_w, k_norm_w, _trace=False,
           **kw):
    nc = _get_nc()
    in_maps = _host_prep(x, cos, sin, Wq, Wk, Wv, Wo, q_norm_w, k_norm_w)
    res = run_bass_kernel_spmd(nc, in_maps, list(range(N_CORES)), trace=_trace,
                               **kw)
    out = np.concatenate([res.results[c]["out_rows"] for c in range(N_CORES)],
                         axis=0)
    out = out.reshape(B, S, D).astype(np.float32)
    if _trace:
        return out, res
    return out



# revision 20
# speedup vs baseline: 1.1418x; 1.1418x over previous
"""GQA attention kernel for 8 trn2 NeuronCores.

Sharding: tensor-parallel over the 8 KV groups (1 group = 4 Q heads per
core, both batch elements), then an AllToAll reshards the per-core
context into row-shards so the output projection runs row-parallel with
no reduction.  Denominators ride the AllToAll and the softmax divide
happens on the destination core.

Shapes (hardcoded): B=2, S=2048, D=2048, H=32, G=8, HD=64.
"""

import numpy as np
import concourse.bass as bass
import concourse.mybir as mybir
import concourse.tile as tile
from concourse import bacc
from concourse.bass import broadcast_tensor_aps
from concourse.bass_utils import run_bass_kernel_spmd
from concourse.masks import make_identity

N_CORES = 8
B, S, D = 2, 2048, 2048
H, G, HD = 32, 8, 64
GS = H // G                       # 4 q heads per kv group
ROWS = B * S                      # 4096 flattened (b, s) rows
RPC = ROWS // N_CORES             # 512 output rows per core
EPS = 1e-6
F32 = mybir.dt.float32
BF16 = mybir.dt.bfloat16
AX = mybir.AxisListType
ALU = mybir.AluOpType
AF = mybir.ActivationFunctionType

KB = D // 128                     # 16 contraction blocks for projections
MB = ROWS // 128                  # 32 row blocks
MBB = MB // B                     # 16 row blocks per batch
QKV = GS * HD + 2 * HD            # 384 projected features per core
NQK = GS + 1                      # 5 heads that get rmsnorm+rope (4 q + 1 k)
NQKD = NQK * HD                   # 320
SQT = 512                         # attention query-tile width
SKT = 128                         # attention key-tile height
JQ = S // SQT                     # 4 query tiles per batch
HF = HD // 2                      # 32 (rope half)
A2F = 2 * (HD + 1)                # 130 rows per a2a slot (2 heads x (64+denom))

_DEBUG = False


def _build():
    nc = bacc.Bacc(num_devices=N_CORES)

    xT = nc.dram_tensor("xT", [D, ROWS], BF16, kind="ExternalInput")
    wqkv = nc.dram_tensor("wqkv", [D, QKV], BF16, kind="ExternalInput")
    wo = nc.dram_tensor("wo", [H * HD, D], BF16, kind="ExternalInput")
    cs = nc.dram_tensor("cs", [S, HD], BF16, kind="ExternalInput")
    sn = nc.dram_tensor("sn", [S, HD], BF16, kind="ExternalInput")
    wvec = nc.dram_tensor("wvec", [1, NQKD], BF16, kind="ExternalInput")
    maskM = nc.dram_tensor("maskM", [128, 1024], BF16, kind="ExternalInput")
    out_rows = nc.dram_tensor("out_rows", [RPC, D], BF16, kind="ExternalOutput")

    with tile.TileContext(nc) as tc:
        with (
            tc.tile_pool(name="const", bufs=1) as const,
            tc.tile_pool(name="dram", bufs=1, space="DRAM") as dram,
        ):
            a2a_in01 = dram.tile([N_CORES, A2F, RPC], BF16)
            a2a_out01 = dram.tile([N_CORES, A2F, RPC], BF16)
            a2a_in23 = dram.tile([N_CORES, A2F, RPC], BF16)
            a2a_out23 = dram.tile([N_CORES, A2F, RPC], BF16)
            rscr = [dram.tile([2 * N_CORES, RPC], BF16, name=f"rscr{i}")
                    for i in range(2)]

            w_sb = const.tile([128, KB, QKV], BF16)
            for k in range(KB):
                nc.sync.dma_start(w_sb[:, k, :], wqkv[k * 128:(k + 1) * 128, :])
            cos_sb = const.tile([128, S // 128, HD], BF16)
            sin_sb = const.tile([128, S // 128, HD], BF16)
            nc.sync.dma_start(cos_sb[:], cs[:].rearrange("(m p) d -> p m d", p=128))
            nc.sync.dma_start(sin_sb[:], sn[:].rearrange("(m p) d -> p m d", p=128))
            mask_sb = const.tile([128, 1024], BF16)
            nc.sync.dma_start(mask_sb[:], maskM[:])
            ident = const.tile([128, 128], BF16)
            make_identity(nc, ident)
            wv1 = const.tile([1, NQKD], BF16)
            nc.sync.dma_start(wv1[:], wvec[:])
            wv_sb = const.tile([128, NQKD], BF16)
            nc.gpsimd.partition_broadcast(wv_sb[:], wv1[0:1, :])
            eps_sb = const.tile([128, 1], F32)
            nc.vector.memset(eps_sb[:], EPS)

            # persistent activations (transposed q/k, row-major v + ones)
            qT_a = [const.tile([128, S], BF16, name=f"qT_a{bb}") for bb in range(B)]
            qT_b = [const.tile([128, S], BF16, name=f"qT_b{bb}") for bb in range(B)]
            kT_t = [const.tile([128, S], BF16, name=f"kT{bb}") for bb in range(B)]
            v1_t = [const.tile([128, S // 128, 2 * HD], BF16, name=f"v1{bb}")
                    for bb in range(B)]
            for bb in range(B):
                nc.vector.memset(v1_t[bb][:], 1.0)  # cols 64:128 stay 1.0 (denominator)

            # PSUM budget: psc 2x2 + pctx 2 + pp 1 + tq 1 = 8 banks
            with (
                tc.tile_pool(name="ps", bufs=2, space="PSUM") as pspool,
                tc.tile_pool(name="pc", bufs=2, space="PSUM") as pcpool,
                tc.tile_pool(name="xs", bufs=20) as xs,
                tc.tile_pool(name="pb", bufs=20) as pbp,
                tc.tile_pool(name="ev", bufs=3) as ev,
                tc.tile_pool(name="ex", bufs=6) as ex,
                tc.tile_pool(name="st", bufs=4) as stp,
                tc.tile_pool(name="cx", bufs=1) as cxp,
                tc.tile_pool(name="ws", bufs=12) as ws,
                tc.tile_pool(name="ou", bufs=3) as ou,
            ):
                from contextlib import ExitStack
                pstack = ExitStack()
                ppool = pstack.enter_context(
                    tc.tile_pool(name="pp", bufs=1, space="PSUM"))
                tpool = pstack.enter_context(
                    tc.tile_pool(name="tp", bufs=1, space="PSUM"))

                # ---------- phase 1: qkv projection + rmsnorm + rope.
                # Two passes per batch so the rsqrt runs as ONE Sqrt
                # activation per batch (avoids ACT table-set thrash
                # against the attention exps).
                for bb in range(B):
                    ppss = []
                    ssb = const.tile([128, MBB, NQK], F32, name=f"ssb{bb}")
                    for m4 in range(MBB // 4):
                        xts = []
                        for k in range(KB):
                            t = xs.tile([128, 512], BF16, tag="xts")
                            nc.sync.dma_start(
                                t[:], xT[k * 128:(k + 1) * 128,
                                         bb * S + m4 * 512:bb * S + (m4 + 1) * 512])
                            xts.append(t)
                        for i in range(4):
                            sm2 = m4 * 4 + i
                            pp = ppool.tile([128, QKV], F32, tag="pp")
                            for k in range(KB):
                                nc.tensor.matmul(
                                    pp[:], xts[k][:, i * 128:(i + 1) * 128],
                                    w_sb[:, k, :], start=(k == 0), stop=(k == KB - 1))
                            pps = pbp.tile([128, QKV], BF16, tag="pps",
                                           name=f"pps{bb}_{sm2}")
                            nc.vector.tensor_copy(pps[:], pp[:])
                            nc.vector.tensor_copy(v1_t[bb][:, sm2, 0:HD],
                                                  pps[:, NQKD:QKV])
                            sq = ev.tile([128, NQKD], BF16, tag="sq")
                            nc.vector.tensor_mul(sq[:], pps[:, :NQKD], pps[:, :NQKD])
                            nc.vector.tensor_reduce(
                                ssb[:, sm2, :],
                                sq[:].rearrange("p (h d) -> p h d", d=HD),
                                AX.X, ALU.add)
                            ppss.append(pps)
                    # batched rsqrt: one Sqrt activation + one reciprocal
                    srt = const.tile([128, MBB, NQK], F32, name=f"srt{bb}")
                    nc.scalar.activation(
                        srt[:].rearrange("p m h -> p (m h)"),
                        ssb[:].rearrange("p m h -> p (m h)"),
                        AF.Sqrt, bias=eps_sb[:], scale=1.0 / HD)
                    rsb = const.tile([128, MBB, NQK], F32, name=f"rsb{bb}")
                    nc.vector.reciprocal(rsb[:].rearrange("p m h -> p (m h)"),
                                         srt[:].rearrange("p m h -> p (m h)"))
                    for sm2 in range(MBB):
                        pps = ppss[sm2]
                        qkn = ev.tile([128, NQKD], BF16, tag="qkn")
                        av, bv = broadcast_tensor_aps(
                            pps[:, :NQKD].rearrange("p (h d) -> p h d", d=HD),
                            rsb[:, sm2, :].rearrange("p (h o) -> p h o", o=1))
                        nc.vector.tensor_tensor(
                            qkn[:].rearrange("p (h d) -> p h d", d=HD), av, bv,
                            ALU.mult)
                        qkw = ev.tile([128, NQKD], BF16, tag="qkw")
                        nc.vector.tensor_mul(qkw[:], qkn[:], wv_sb[:])
                        # rope (rotate-half) on all 5 heads at once, bf16
                        qv = qkw[:].rearrange("p (h d) -> p h d", d=HD)
                        qkr = ev.tile([128, NQKD], BF16, tag="qkr")
                        rv = qkr[:].rearrange("p (h d) -> p h d", d=HD)
                        t1 = ev.tile([128, NQK, HF], BF16, tag="t1")
                        t2 = ev.tile([128, NQK, HF], BF16, tag="t2")

                        def bmul(out_ap, a_ap, trig, lo):
                            tr = trig[:, sm2:sm2 + 1, lo * HF:(lo + 1) * HF]
                            av2, bv2 = broadcast_tensor_aps(a_ap, tr)
                            nc.vector.tensor_tensor(out_ap, av2, bv2, ALU.mult)

                        bmul(t1[:], qv[:, :, HF:HD], sin_sb, 0)   # x2 * sin_lo
                        bmul(t2[:], qv[:, :, 0:HF], sin_sb, 1)    # x1 * sin_hi
                        bmul(rv[:, :, 0:HF], qv[:, :, 0:HF], cos_sb, 0)
                        bmul(rv[:, :, HF:HD], qv[:, :, HF:HD], cos_sb, 1)
                        nc.vector.tensor_sub(rv[:, :, 0:HF], rv[:, :, 0:HF], t1[:])
                        nc.vector.tensor_add(rv[:, :, HF:HD], rv[:, :, HF:HD], t2[:])
                        # transposes: [seq,hd] -> [hd,seq]; k duplicated to
                        # both partition halves for the packed QK row-groups
                        cl = slice(sm2 * 128, (sm2 + 1) * 128)
                        tq1 = tpool.tile([128, 128], BF16, tag="tq")
                        nc.tensor.transpose(tq1[:], qkr[:, 0:128], ident[:])
                        nc.vector.tensor_copy(qT_a[bb][:, cl], tq1[:])
                        tq2 = tpool.tile([128, 128], BF16, tag="tq")
                        nc.tensor.transpose(tq2[:], qkr[:, 128:256], ident[:])
                        nc.vector.tensor_copy(qT_b[bb][:, cl], tq2[:])
                        tqk = tpool.tile([64, 128], BF16, tag="tq")
                        nc.tensor.transpose(tqk[:], qkr[:, 256:320], ident[:])
                        nc.vector.tensor_copy(kT_t[bb][0:64, cl], tqk[:])
                        nc.vector.tensor_copy(kT_t[bb][64:128, cl], tqk[:])

                pstack.close()  # free proj PSUM banks for the out-proj pool
                popool_cm = tc.tile_pool(name="po", bufs=2, space="PSUM")
                popool = popool_cm.__enter__()
                o1p_cm = tc.tile_pool(name="o1p", bufs=16)
                o1p = o1p_cm.__enter__()

                # ---------- phase 2: attention (overlaps phase 1)
                # QK for the head pair runs as two row-group-packed matmuls
                # (tile_position (0,0)/(64,0)) into one 2-bank score tile so
                # they sit adjacent in the PE queue and the exp covers both.
                def attention(p, b):
                    qT_t = qT_a[b] if p == 0 else qT_b[b]
                    for jq in range(JQ):
                        q_e = qT_t[0:64, jq * SQT:(jq + 1) * SQT]
                        q_o = qT_t[64:128, jq * SQT:(jq + 1) * SQT]
                        nkb = (jq + 1) * (SQT // SKT)
                        ess = []
                        pce = pcpool.tile([128, SQT], F32, tag="pctx",
                                          name=f"pce{p}_{b}_{jq}")
                        pco = pcpool.tile([128, SQT], F32, tag="pctx",
                                          name=f"pco{p}_{b}_{jq}")

                        def pv(j, last):
                            es = ess[j]
                            nc.tensor.matmul(pce[:], v1_t[b][:, j, :],
                                             es[:, 0:SQT],
                                             start=(j == 0), stop=last)
                            nc.tensor.matmul(pco[:], v1_t[b][:, j, :],
                                             es[:, SQT:2 * SQT],
                                             start=(j == 0), stop=last)

                        for ik in range(nkb):
                            psc = pspool.tile([128, 2 * SQT], F32, tag="psc",
                                              name=f"psc{ik}")
                            k_e = kT_t[b][0:64, ik * SKT:(ik + 1) * SKT]
                            k_o = kT_t[b][64:128, ik * SKT:(ik + 1) * SKT]
                            nc.tensor.matmul(psc[:, 0:SQT], k_e, q_e,
                                             start=True, stop=True,
                                             tile_position=(0, 0))
                            nc.tensor.matmul(psc[:, SQT:2 * SQT], k_o, q_o,
                                             start=True, stop=True,
                                             tile_position=(64, 0))
                            es = ex.tile([128, 2 * SQT], BF16, tag="es",
                                         name=f"es{ik}")
                            nc.scalar.activation(es[:], psc[:], AF.Exp,
                                                 scale=1.0 / np.sqrt(HD))
                            dd = ik * SKT - jq * SQT
                            if dd >= 0:
                                off = 512 - dd
                                mv = mask_sb[:, off:off + SQT]
                                nc.vector.tensor_mul(es[:, 0:SQT],
                                                     es[:, 0:SQT], mv)
                                nc.vector.tensor_mul(es[:, SQT:2 * SQT],
                                                     es[:, SQT:2 * SQT], mv)
                            ess.append(es)
                            # lag-1 PV keeps the PE queue from stalling
                            # behind the exp of the same block
                            if ik > 0:
                                pv(ik - 1, False)
                        pv(nkb - 1, True)
                        ste = stp.tile([HD + 1, SQT], BF16, tag="st")
                        nc.vector.tensor_copy(ste[:], pce[0:HD + 1, :])
                        sto = stp.tile([HD + 1, SQT], BF16, tag="st")
                        nc.vector.tensor_copy(sto[:], pco[0:HD + 1, :])
                        dst = a2a_in01 if p == 0 else a2a_in23
                        slot = b * JQ + jq
                        nc.sync.dma_start(dst[slot, 0:HD + 1, :], ste[:])
                        nc.sync.dma_start(dst[slot, HD + 1:A2F, :], sto[:])

                def load_cx(a2a_out, parity):
                    tiles = {}
                    for g in range(N_CORES):
                        t = cxp.tile([128, RPC], BF16, tag=f"cx{2 * g + parity}",
                                     name=f"cx{2 * g + parity}")
                        nc.sync.dma_start(t[0:HD, :], a2a_out[g, 0:HD, :])
                        nc.sync.dma_start(t[HD:128, :],
                                          a2a_out[g, HD + 1:A2F - 1, :])
                        tiles[g] = t
                    dn = cxp.tile([2 * N_CORES, RPC], BF16, tag=f"dn{parity}",
                                  name=f"dn{parity}")
                    for g in range(N_CORES):
                        for h in range(2):
                            i = 2 * g + h
                            nc.sync.dma_start(
                                dn[i:i + 1, :],
                                a2a_out[g, 65 * h + 64:65 * h + 65, :])
                    rinv = cxp.tile([2 * N_CORES, RPC], BF16, tag=f"rinv{parity}",
                                    name=f"rinv{parity}")
                    with nc.allow_low_precision(
                            reason="softmax denom reciprocal shipped bf16"):
                        nc.vector.reciprocal(rinv[:], dn[:])
                    # DVE can't broadcast across partitions: bounce the
                    # reciprocals through DRAM and reload with a stride-0
                    # partition-replicating DMA pattern
                    nc.sync.dma_start(rscr[parity][:], rinv[:])
                    for g in range(N_CORES):
                        rf = stp.tile([128, RPC], BF16, tag="rf")
                        for h in range(2):
                            i = 2 * g + h
                            nc.sync.dma_start(
                                rf[h * HD:(h + 1) * HD, :],
                                rscr[parity][i:i + 1, :].partition_broadcast(HD))
                        nc.vector.tensor_mul(tiles[g][:], tiles[g][:], rf[:])
                    return tiles

                NH = KB // 2

                def outproj(cxt, parity, o1s):
                    for n in range(D // 512):
                        wts = []
                        for g in range(NH):
                            wt = ws.tile([128, 512], BF16, tag="wt",
                                         name=f"wt{parity}_{n}_{g}")
                            k = 2 * g + parity
                            nc.sync.dma_start(
                                wt[:],
                                wo[k * 128:(k + 1) * 128, n * 512:(n + 1) * 512])
                            wts.append(wt)
                        for mi in range(4):
                            pj = popool.tile([128, 512], F32, tag="po",
                                             name=f"pj{parity}_{n}_{mi}")
                            for g in range(NH):
                                nc.tensor.matmul(
                                    pj[:], cxt[g][:, mi * 128:(mi + 1) * 128],
                                    wts[g][:], start=(g == 0), stop=(g == NH - 1))
                            if parity == 0:
                                o1 = o1p.tile([128, 512], F32, tag="o1",
                                              name=f"o1_{n}_{mi}")
                                nc.vector.tensor_copy(o1[:], pj[:])
                                o1s[(n, mi)] = o1
                            else:
                                ot = ou.tile([128, 512], BF16, tag="ot",
                                             name=f"ot{n}_{mi}")
                                nc.vector.tensor_add(ot[:], pj[:],
                                                     o1s[(n, mi)][:])
                                nc.sync.dma_start(
                                    out_rows[mi * 128:(mi + 1) * 128,
                                             n * 512:(n + 1) * 512],
                                    ot[:])

                for b in range(B):
                    attention(0, b)
                nc.gpsimd.collective_compute(
                    "AllToAll", ALU.bypass,
                    replica_groups=[list(range(N_CORES))],
                    ins=[a2a_in01.opt()], outs=[a2a_out01.opt()])
                for b in range(B):
                    attention(1, b)
                # cx01 unpack + normalize runs during pair-1 attention; the
                # even out-proj then fills the PE while AllToAll #2 runs
                cx01 = load_cx(a2a_out01, 0)
                nc.gpsimd.collective_compute(
                    "AllToAll", ALU.bypass,
                    replica_groups=[list(range(N_CORES))],
                    ins=[a2a_in23.opt()], outs=[a2a_out23.opt()])
                o1s = {}
                outproj(cx01, 0, o1s)
                cx23 = load_cx(a2a_out23, 1)
                outproj(cx23, 1, o1s)
                popool_cm.__exit__(None, None, None)
                o1p_cm.__exit__(None, None, None)

    nc.finalize()
    return nc


_NC_CACHE = None


def _get_nc():
    global _NC_CACHE
    if _NC_CACHE is None:
        _NC_CACHE = _build()
    return _NC_CACHE


def _host_prep(x, cos, sin, Wq, Wk, Wv, Wo, q_norm_w, k_norm_w):
    import ml_dtypes
    BF = ml_dtypes.bfloat16
    xT = np.ascontiguousarray(
        np.asarray(x, np.float32).transpose(2, 0, 1).reshape(D, ROWS).astype(BF))
    f = np.arange(1024)[None, :]
    p = np.arange(128)[:, None]
    maskM = (p + 512 <= f).astype(BF)
    wvec = np.concatenate(
        [np.tile(np.asarray(q_norm_w, np.float32), GS),
         np.asarray(k_norm_w, np.float32)]).reshape(1, NQKD).astype(BF)
    base = dict(
        cs=np.ascontiguousarray(np.asarray(cos, np.float32).astype(BF)),
        sn=np.ascontiguousarray(np.asarray(sin, np.float32).astype(BF)),
        maskM=maskM,
        wvec=np.ascontiguousarray(wvec),
        xT=xT,
    )
    wo_c = np.ascontiguousarray(np.asarray(Wo, np.float32).astype(BF))
    in_maps = []
    for c in range(N_CORES):
        wqkv = np.concatenate(
            [np.asarray(Wq, np.float32)[:, c * GS * HD:(c + 1) * GS * HD],
             np.asarray(Wk, np.float32)[:, c * HD:(c + 1) * HD],
             np.asarray(Wv, np.float32)[:, c * HD:(c + 1) * HD]], axis=1)
        in_maps.append(dict(base, wqkv=np.ascontiguousarray(wqkv.astype(BF)),
                            wo=wo_c))
    return in_maps


def kernel(x, mask, cos, sin, Wq, Wk, Wv, Wo, q_norm_w, k_norm_w, _trace=False,
           **kw):
    nc = _get_nc()
    in_maps = _host_prep(x, cos, sin, Wq, Wk, Wv, Wo, q_norm_w, k_norm_w)
    res = run_bass_kernel_spmd(nc, in_maps, list(range(N_CORES)), trace=_trace,
                               **kw)
    out = np.concatenate([res.results[c]["out_rows"] for c in range(N_CORES)],
                         axis=0)
    out = out.reshape(B, S, D).astype(np.float32)
    if _trace:
        return out, res
    return out
